# revision 1
# baseline (speedup 1.0000x reference)
"""MoE model (embed -> gate -> 4 dense experts -> softmax combine) on 8 TRN2 cores.

Data-parallel: batch (65536 tokens) sharded 8192/core; expert/gating weights
replicated on every core (SBUF-resident, bf16). All on-chip activations are
kept feature-major ("transposed") so that every matmul consumes operands in
their natural layout:

  e_T[f, t]   = embedding lookup, feature-major, via transposing gather DMAs
                issued one supertile ahead on the otherwise-idle GpSimd SWDGE
                path (fallback: one-hot-mask matmul on the PE).
  h_T[d, t]   = silu(W1[e].T-tiles @ e_T + b1)       (PSUM fp32, evac bf16)
  eo_T[o, t]  = W2[e].T-tiles @ h_T + b2             (PSUM fp32)
  logits[e,t] = Wg.T-tiles @ e_T + bg ; softmax via exp / sum (unnormalized
                weights combined first, one reciprocal row scale at the end)
  out_T[o, t] = (sum_e exp_e * eo_e) * recip         (DVE, fp32)

Output per core is [128, 8192] (feature-major); host transposes on unshard.

bf16 inputs with fp32 PSUM accumulation: end-to-end relative error vs the
fp32 reference is ~0.5%.
"""

import os
import numpy as np
import ml_dtypes

import concourse.bass as bass
import concourse.mybir as mybir
import concourse.tile as tile
from concourse.bass_utils import run_bass_kernel_spmd

BF16 = ml_dtypes.bfloat16

B = 65536
V = 512
D = 1024
IN = 2048
E = 4
OUT = 128
NCORES = 8
BL = B // NCORES          # tokens per core
ST = 512                  # tokens per supertile (max PSUM free dim, fp32)
NST = BL // ST            # supertiles per core
KC = IN // 128            # 16 feature chunks
DC = D // 128             # 8 hidden chunks
VC = V // 128             # 4 vocab chunks

LAST_EXEC_NS = None       # set when BASSMOE_TRACE=1


def _legalize_waits(nc, max_waits=1):
    """This walrus build rejects instructions carrying more than ~1 sync-wait
    command ("Too many sync wait commands", CoreV2/V3GenImpl setupSyncWait).
    Hoist all but the last wait of every instruction onto single-wait NoOps
    placed immediately before it in the same engine's stream."""
    for f in nc.m.functions:
        for bb in f.blocks:
            insts = bb.instructions
            if not any(
                inst.sync_info is not None and len(inst.sync_info.on_wait) > max_waits
                for inst in insts
            ):
                continue
            new = []
            for inst in insts:
                si = inst.sync_info
                waits = list(si.on_wait) if si is not None else []
                if len(waits) > max_waits:
                    for w in waits[:-max_waits]:
                        nop = mybir.InstNoOp(
                            name=f"legw-{nc.next_id()}", ins=[], outs=[]
                        )
                        nop.engine = inst.engine
                        nop.sync_info = mybir.SyncInfo(on_wait=[w], on_update=[])
                        new.append(nop)
                    inst.sync_info = mybir.SyncInfo(
                        on_wait=waits[-max_waits:], on_update=list(si.on_update)
                    )
                new.append(inst)
            bb.instructions = new


def build_program(nst=NST, legalize=True, n_gather=2):
    """n_gather: how many of the 2 embedding tables use the gather-DMA path
    (the rest use the one-hot matmul path)."""
    dt = mybir.dt
    f32, bf16, f16 = dt.float32, dt.bfloat16, dt.float16
    AF = mybir.ActivationFunctionType
    ALU = mybir.AluOpType

    gathered = [t < n_gather for t in range(2)]
    n_onehot = 2 - n_gather

    nc = bass.Bass()

    xd = [None, None]
    for t in range(2):
        if gathered[t]:
            # wrapped gather-idx layout: idx j at [j%16, j//16], replicated
            # across the 8 gpsimd cores
            xd[t] = nc.dram_tensor(
                f"x{t}i", [nst, 128, ST // 16], dt.int16, kind="ExternalInput"
            )
        else:
            xd[t] = nc.dram_tensor(
                f"x{t}", [nst, 1, ST], f16, kind="ExternalInput"
            )
    if n_gather:
        embgd = nc.dram_tensor("embg", [n_gather, V, D], bf16, kind="ExternalInput")
    if n_onehot:
        embd = nc.dram_tensor(
            "embs", [128, n_onehot, VC, DC, 128], bf16, kind="ExternalInput"
        )
        ivd = nc.dram_tensor("ivs", [128, VC], f32, kind="ExternalInput")
    w1d = nc.dram_tensor("w1s", [E, 128, KC, DC, 128], bf16, kind="ExternalInput")
    w2d = nc.dram_tensor("w2s", [128, E, DC, OUT], bf16, kind="ExternalInput")
    wgd = nc.dram_tensor("wgs", [128, KC, E], bf16, kind="ExternalInput")
    b1d = nc.dram_tensor("b1s", [128, E, DC], f32, kind="ExternalInput")
    b2d = nc.dram_tensor("b2s", [128, E], f32, kind="ExternalInput")
    bgd = nc.dram_tensor("bgs", [E, 1], f32, kind="ExternalInput")
    seld = nc.dram_tensor("sels", [E, E, 128], bf16, kind="ExternalInput")
    outd = nc.dram_tensor("out", [128, nst * ST], f32, kind="ExternalOutput")

    with tile.TileContext(nc) as tc:
        with (
            tc.tile_pool(name="const", bufs=1) as cpool,
            tc.tile_pool(name="xt", bufs=2) as xpool,
            tc.tile_pool(name="mask", bufs=1) as mpool,
            tc.tile_pool(name="etg", bufs=2) as etgpool,
            tc.tile_pool(name="et", bufs=1) as etpool,
            tc.tile_pool(name="hs", bufs=1) as hpool,
            tc.tile_pool(name="sm", bufs=2) as smpool,
            tc.tile_pool(name="gsc", bufs=1) as gspool,
            tc.tile_pool(name="sgp", bufs=2) as sgpool,
            tc.tile_pool(name="accp", bufs=2) as apool,
            tc.tile_pool(name="outp", bufs=2) as opool,
            tc.tile_pool(name="pmm", bufs=2, space="PSUM") as pmm,
            tc.tile_pool(name="peo", bufs=2, space="PSUM") as peo,
            tc.tile_pool(name="prb", bufs=2, space="PSUM") as prb,
            tc.tile_pool(name="pmisc", bufs=2, space="PSUM") as pmisc,
        ):
            # --- prologue: supertile 0's embedding inputs first ---
            if n_gather:
                from concourse import library_config

                nc.gpsimd.load_library(library_config.mlp)

                def issue_gather(i, t):
                    """table t embedding rows for supertile i -> feature-major
                    e_T chunk tile, via the GpSimd transposing gather DMA."""
                    xi = xpool.tile([128, ST // 16], dt.int16, tag=f"xi{t}")
                    nc.sync.dma_start(xi[:], xd[t][i])
                    etg = etgpool.tile([128, DC, ST], bf16, tag=f"eTg{t}")
                    nc.gpsimd.dma_gather(
                        out_ap=etg[:],
                        in_ap=embgd[t],
                        idxs_ap=xi[:],
                        num_idxs=ST,
                        num_idxs_reg=ST,
                        elem_size=D,
                        transpose=True,
                    )
                    return etg

            if n_onehot:
                iv_sb = cpool.tile([128, VC], f32)
                nc.sync.dma_start(iv_sb[:], ivd[:])
                ones_f16 = cpool.tile([1, 128], f16)
                nc.vector.memset(ones_f16[:], 1.0)
                x0_pre = []
                for t in range(2):
                    if not gathered[t]:
                        xs = xpool.tile([1, ST], f16, tag=f"x{t}")
                        nc.sync.dma_start(xs[:], xd[t][0])
                        x0_pre.append(xs)
                emb_sb = cpool.tile([128, n_onehot, VC, DC, 128], bf16)
                nc.sync.dma_start(emb_sb[:], embd[:])

            cur_etg = [issue_gather(0, t) if gathered[t] else None for t in range(2)]

            # --- resident weights (DMA queue order = when they are needed) ---
            wg_sb = cpool.tile([128, KC, E], bf16)
            nc.sync.dma_start(wg_sb[:], wgd[:])
            b1_sb = cpool.tile([128, E, DC], f32)
            nc.sync.dma_start(b1_sb[:], b1d[:])
            b2_sb = cpool.tile([128, E], f32)
            nc.sync.dma_start(b2_sb[:], b2d[:])
            bg_sb = cpool.tile([E, 1], f32)
            nc.sync.dma_start(bg_sb[:], bgd[:])
            sel_sb = cpool.tile([E, E, 128], bf16)
            nc.sync.dma_start(sel_sb[:], seld[:])
            w1_sbs = []
            for e in range(E):
                t = cpool.tile([128, KC, DC, 128], bf16, tag=f"w1e{e}")
                w1_sbs.append(t)
            nc.sync.dma_start(w1_sbs[0][:], w1d[0])
            w2_sb = cpool.tile([128, E, DC, OUT], bf16)
            nc.sync.dma_start(w2_sb[:], w2d[:])
            for e in range(1, E):
                nc.sync.dma_start(w1_sbs[e][:], w1d[e])

            ones4_bf = cpool.tile([E, 1], bf16)
            nc.vector.memset(ones4_bf[:], 1.0)
            ones128_bf = cpool.tile([1, 128], bf16)
            nc.vector.memset(ones128_bf[:], 1.0)

            def build_masks(i, preloaded=None):
                """x-broadcast (K=1 matmul) + one-hot compares for the
                one-hot-embedded tables of supertile i."""
                ms = {}
                pi = 0
                for t in range(2):
                    if gathered[t]:
                        continue
                    if preloaded is None:
                        xs = xpool.tile([1, ST], f16, tag=f"x{t}")
                        nc.sync.dma_start(xs[:], xd[t][i])
                    else:
                        xs = preloaded[pi]
                        pi += 1
                    p = pmisc.tile([128, ST], f32, tag="misc")
                    nc.tensor.matmul(p[:], ones_f16[:], xs[:])
                    row = []
                    for vc in range(VC):
                        m = mpool.tile([128, ST], bf16, tag=f"m{t}{vc}")
                        nc.vector.tensor_scalar(
                            m[:], p[:], iv_sb[:, vc : vc + 1], None, ALU.is_equal
                        )
                        row.append(m)
                    ms[t] = row
                return ms

            cur_masks = build_masks(0, preloaded=x0_pre) if n_onehot else {}

            for i in range(nst):
                # --- one-hot embedding matmul -> e_T (one-hot tables) ---
                if n_onehot:
                    eT = etpool.tile([128, n_onehot, DC, ST], bf16, tag="eT")
                    oh = 0
                    for t in range(2):
                        if gathered[t]:
                            continue
                        for dc in range(DC):
                            ps = pmm.tile([128, ST], f32, tag="mm")
                            for vc in range(VC):
                                nc.tensor.matmul(
                                    ps[:],
                                    emb_sb[:, oh, vc, dc, :],
                                    cur_masks[t][vc][:],
                                    start=(vc == 0),
                                    stop=(vc == VC - 1),
                                )
                            nc.scalar.copy(eT[:, oh, dc, :], ps[:])
                        oh += 1

                oh_index = {}
                oh = 0
                for t in range(2):
                    if not gathered[t]:
                        oh_index[t] = oh
                        oh += 1

                def eT_chunk(kc):
                    t, dc = kc // DC, kc % DC
                    if gathered[t]:
                        return cur_etg[t][:, dc, :]
                    return eT[:, oh_index[t], dc, :]

                # --- gating: logits -> exp -> sum -> reciprocal bcast ---
                lp = pmisc.tile([E, ST], f32, tag="misc")
                for kc in range(KC):
                    nc.tensor.matmul(
                        lp[:],
                        wg_sb[:, kc, :],
                        eT_chunk(kc),
                        start=(kc == 0),
                        stop=(kc == KC - 1),
                    )
                expt = smpool.tile([E, ST], bf16, tag="expt")
                nc.scalar.activation(expt[:], lp[:], AF.Exp, bias=bg_sb[:])

                def emit_recip_chain():
                    # sum-exp -> reciprocal -> bf16 -> broadcast to 128 rows.
                    # Emitted between expert 0 and 1 so the slow single-
                    # partition RECIPROCAL (~3.3us DVE) and the Exp/Sigmoid
                    # ACT-table switch hide under expert-0's W1 matmuls
                    # instead of stalling the PE at the supertile boundary.
                    sp = pmisc.tile([1, ST], f32, tag="misc")
                    nc.tensor.matmul(sp[:], ones4_bf[:], expt[:])
                    rec = smpool.tile([1, ST], f32, tag="rec")
                    nc.vector.reciprocal(rec[:], sp[:])
                    recb = smpool.tile([1, ST], bf16, tag="recb")
                    nc.vector.tensor_copy(recb[:], rec[:])
                    rbp = prb.tile([128, ST], f32, tag="rb")
                    nc.tensor.matmul(rbp[:], ones128_bf[:], recb[:])
                    return rbp

                # prefetch next supertile's embeddings: gather DMAs + mask
                # compares overlap with the expert phase below
                next_etg = [None, None]
                if i + 1 < nst:
                    for t in range(2):
                        if gathered[t]:
                            next_etg[t] = issue_gather(i + 1, t)
                    next_masks = build_masks(i + 1) if n_onehot else {}

                # --- experts ---
                acc = apool.tile([128, ST], f32, tag="acc")
                for e in range(E):
                    if e == 1:
                        rbp = emit_recip_chain()
                    # hs as per-chunk tiles: W2's dc-th matmul then only waits
                    # for the dc-th silu chunk, not the whole expert's h
                    hs = []
                    for dc in range(DC):
                        hp = pmm.tile([128, ST], f32, tag="mm")
                        for kc in range(KC):
                            nc.tensor.matmul(
                                hp[:],
                                w1_sbs[e][:, kc, dc, :],
                                eT_chunk(kc),
                                start=(kc == 0),
                                stop=(kc == KC - 1),
                            )
                        sg = sgpool.tile([128, ST], f32, tag="sg")
                        nc.scalar.activation(
                            sg[:], hp[:], AF.Sigmoid, bias=b1_sb[:, e, dc : dc + 1]
                        )
                        h_dc = hpool.tile([128, ST], bf16, tag=f"hs{dc}")
                        nc.vector.scalar_tensor_tensor(
                            h_dc[:], hp[:], b1_sb[:, e, dc : dc + 1], sg[:],
                            ALU.add, ALU.mult,
                        )
                        hs.append(h_dc)
                    eop = peo.tile([128, ST], f32, tag="eo")
                    for dc in range(DC):
                        nc.tensor.matmul(
                            eop[:],
                            w2_sb[:, e, dc, :],
                            hs[dc][:],
                            start=(dc == 0),
                            stop=(dc == DC - 1),
                        )
                    gp = pmisc.tile([128, ST], f32, tag="misc")
                    nc.tensor.matmul(gp[:], sel_sb[:, e, :], expt[:])
                    gs = gspool.tile([128, ST], f32, tag="gs")
                    nc.scalar.copy(gs[:], gp[:])
                    if e == 0:
                        nc.vector.scalar_tensor_tensor(
                            acc[:], eop[:], b2_sb[:, e : e + 1], gs[:],
                            ALU.add, ALU.mult,
                        )
                    else:
                        tmp = opool.tile([128, ST], f32, tag="outt")
                        nc.vector.scalar_tensor_tensor(
                            tmp[:], eop[:], b2_sb[:, e : e + 1], gs[:],
                            ALU.add, ALU.mult,
                        )
                        nc.vector.tensor_add(acc[:], acc[:], tmp[:])

                outt = opool.tile([128, ST], f32, tag="outt")
                nc.vector.tensor_tensor(outt[:], acc[:], rbp[:], ALU.mult)
                nc.sync.dma_start(outd[:, i * ST : (i + 1) * ST], outt[:])
                if i + 1 < nst:
                    cur_etg = next_etg
                    if n_onehot:
                        cur_masks = next_masks

    if legalize:
        _legalize_waits(nc)
    # populate .instr bytes for extended-ISA instructions (library reload for
    # dma_gather) — raw Bass skips Bacc's codegen pass; walrus errors with
    # "ISA wrong length" on empty instr otherwise
    mybir.codegen_inst_isa_subclasses(nc)
    return nc


def marshal_inputs(
    x, emb0, emb1, W1, b1, W2, b2, Wg, bg, nst=NST, ncores=NCORES, n_gather=2
):
    """Host-side: cast/reshape full inputs into per-core in_maps."""
    n_tok = ncores * nst * ST
    gathered = [t < n_gather for t in range(2)]
    tables = [emb0, emb1]

    def _wrap_idx(col):
        # dma_gather wrapped layout, tiled 8x across partitions (8 Q7 cores)
        w = (
            col[:n_tok].astype(np.int16).reshape(ncores, nst, ST // 16, 16)
            .transpose(0, 1, 3, 2)
        )
        return np.ascontiguousarray(np.tile(w, (1, 1, 8, 1)))

    def _f16_rows(col):
        return np.ascontiguousarray(
            col[:n_tok].astype(np.float16).reshape(ncores, nst, 1, ST)
        )

    shared = {}
    xh = {}
    for t in range(2):
        if gathered[t]:
            xh[f"x{t}i"] = _wrap_idx(x[:, t])
        else:
            xh[f"x{t}"] = _f16_rows(x[:, t])
    if n_gather:
        shared["embg"] = np.ascontiguousarray(
            np.stack([np.asarray(tables[t]) for t in range(2) if gathered[t]]).astype(
                BF16
            )
        )
    if n_gather < 2:
        onehot_tabs = [np.asarray(tables[t]) for t in range(2) if not gathered[t]]
        shared["embs"] = np.ascontiguousarray(
            np.stack(onehot_tabs)
            .reshape(len(onehot_tabs), VC, 128, DC, 128)
            .transpose(2, 0, 1, 3, 4)
            .astype(BF16)
        )
        shared["ivs"] = np.ascontiguousarray(
            (np.arange(VC)[None, :] * 128 + np.arange(128)[:, None]).astype(np.float32)
        )

    shared["w1s"] = np.ascontiguousarray(
        np.asarray(W1).reshape(E, KC, 128, DC, 128).transpose(0, 2, 1, 3, 4).astype(BF16)
    )
    shared["w2s"] = np.ascontiguousarray(
        np.asarray(W2).reshape(E, DC, 128, OUT).transpose(2, 0, 1, 3).astype(BF16)
    )
    shared["wgs"] = np.ascontiguousarray(
        np.asarray(Wg).reshape(KC, 128, E).transpose(1, 0, 2).astype(BF16)
    )
    shared["b1s"] = np.ascontiguousarray(
        np.asarray(b1).reshape(E, DC, 128).transpose(2, 0, 1).astype(np.float32)
    )
    shared["b2s"] = np.ascontiguousarray(np.asarray(b2).T.astype(np.float32))
    shared["bgs"] = np.ascontiguousarray(np.asarray(bg).reshape(E, 1).astype(np.float32))
    shared["sels"] = np.ascontiguousarray(
        np.broadcast_to(np.eye(E, dtype=np.float32)[:, :, None], (E, E, 128)).astype(
            BF16
        )
    )
    return [{**{k: v[c] for k, v in xh.items()}, **shared} for c in range(ncores)]


def kernel(x, emb0, emb1, W1, b1, W2, b2, Wg, bg):
    global LAST_EXEC_NS
    nc = build_program()
    in_maps = marshal_inputs(x, emb0, emb1, W1, b1, W2, b2, Wg, bg)
    trace = os.environ.get("BASSMOE_TRACE", "0") == "1"
    res = run_bass_kernel_spmd(nc, in_maps, list(range(NCORES)), trace=trace)
    LAST_EXEC_NS = res.exec_time_ns
    out = np.empty((B, OUT), dtype=np.float32)
    for c in range(NCORES):
        out[c * BL : (c + 1) * BL, :] = res.results[c]["out"].T
    return out



# revision 2
# speedup vs baseline: 1.0113x; 1.0113x over previous
"""MoE model via vocab-precompute + bucketed one-hot selection on 8 TRN2 cores.

v2 replaced the dense W1 matmul with a per-vocab precompute (T tables) and a
per-token one-hot selection (K=512 per table).  v3 bucketed tokens by their
(i0//128, i1//128) vocab-chunk pair into 16 "pure" supertiles + n_mixed spill
supertiles, cutting selection to one matmul per table per feature chunk.

v4 removes the scalar-engine pacing found in the v3 trace:
  - b1 and bg are folded into the T/G precompute via an extra K=1 matmul row
    (ones stationary, bias as moving), so the silu evac needs no bias AP.
  - silu evacs are PAIRED: pmm tiles span 2 PSUM banks [128, 2, 512]; one
    ACTIVATE covers 1024 columns, amortizing the 352-cycle fixed cost.
  - exp FACTORIZES over the two tables: exp(G0[i0]+G1[i1]+bg) =
    expG0[i0] * expG1[i1].  exp moves into the precompute (G tables stored
    exp'd); the main loop multiplies two one-hot gate selections on the DVE.
    No Exp in the loop -> no ACT table-set switches at all.
  - the combine's gate operand is read straight from its broadcast PSUM.
"""

import os
import numpy as np
import ml_dtypes

import concourse.bass as bass
import concourse.mybir as mybir
import concourse.tile as tile
from concourse.bass_utils import run_bass_kernel_spmd

BF16 = ml_dtypes.bfloat16

B = 65536
V = 512
D = 1024
IN = 2048
E = 4
OUT = 128
NCORES = 8
BL = B // NCORES          # tokens per core
ST = 512                  # tokens per supertile (max PSUM free dim, fp32)
F = E * D                 # 4096 selected features (expert-major)
FC = F // 128             # 32 feature chunks
VC = V // 128             # 4 vocab chunks
KC = D // 128             # 8 contraction chunks per table (precompute)
FB = 4                    # W1 f-blocks streamed (1024 feats each)
FPB = 2                   # 512-wide f-pieces per f-block
NPURE = VC * VC           # 16 pure supertiles (one per chunk pair)

LAST_EXEC_NS = None       # set when BASSMOE_TRACE=1


def _legalize_waits(nc, max_waits=1):
    """This walrus build rejects instructions carrying more than ~1 sync-wait
    command; hoist all but the last wait onto single-wait NoOps."""
    for f in nc.m.functions:
        for bb in f.blocks:
            insts = bb.instructions
            if not any(
                inst.sync_info is not None and len(inst.sync_info.on_wait) > max_waits
                for inst in insts
            ):
                continue
            new = []
            for inst in insts:
                si = inst.sync_info
                waits = list(si.on_wait) if si is not None else []
                if len(waits) > max_waits:
                    for w in waits[:-max_waits]:
                        nop = mybir.InstNoOp(
                            name=f"legw-{nc.next_id()}", ins=[], outs=[]
                        )
                        nop.engine = inst.engine
                        nop.sync_info = mybir.SyncInfo(on_wait=[w], on_update=[])
                        new.append(nop)
                    inst.sync_info = mybir.SyncInfo(
                        on_wait=waits[-max_waits:], on_update=list(si.on_update)
                    )
                new.append(inst)
            bb.instructions = new


def _present(s):
    """Vocab chunks present in supertile s, per table."""
    if s < NPURE:
        return [[s // VC], [s % VC]]
    return [list(range(VC)), list(range(VC))]


def build_program(n_mixed, legalize=True):
    nst = NPURE + n_mixed
    dt = mybir.dt
    f32, bf16, f16 = dt.float32, dt.bfloat16, dt.float16
    AF = mybir.ActivationFunctionType
    ALU = mybir.AluOpType

    nc = bass.Bass()

    x0d = nc.dram_tensor("x0", [nst, 1, ST], f16, kind="ExternalInput")
    x1d = nc.dram_tensor("x1", [nst, 1, ST], f16, kind="ExternalInput")
    # emb pre-transposed: embT[t, kc, p, v] = emb_t[v, kc*128+p]
    embtd = nc.dram_tensor("embt", [2, KC, 128, V], bf16, kind="ExternalInput")
    # W1 re-laid: w1m[t, fb, kc, p, ff] = W1flat[t*1024+kc*128+p, fb*1024+ff]
    w1d = nc.dram_tensor("w1m", [2, FB, KC, 128, 1024], bf16, kind="ExternalInput")
    b1rd = nc.dram_tensor("b1row", [1, F], bf16, kind="ExternalInput")
    bgrd = nc.dram_tensor("bgrow", [1, E], bf16, kind="ExternalInput")
    wgd = nc.dram_tensor("wgm", [128, 2, KC, E], bf16, kind="ExternalInput")
    w2d = nc.dram_tensor("w2s", [128, E, KC, OUT], bf16, kind="ExternalInput")
    b2d = nc.dram_tensor("b2s", [128, E], f32, kind="ExternalInput")
    seld = nc.dram_tensor("sels", [E, E, 128], bf16, kind="ExternalInput")
    ivd = nc.dram_tensor("ivs", [128, VC], f32, kind="ExternalInput")
    outd = nc.dram_tensor("out", [128, nst * ST], f32, kind="ExternalOutput")

    with tile.TileContext(nc) as tc:
        with (
            tc.tile_pool(name="const", bufs=1) as cpool,
            tc.tile_pool(name="w1st", bufs=2) as w1pool,
            tc.tile_pool(name="xt", bufs=2) as xpool,
            tc.tile_pool(name="mask", bufs=2) as mpool,
            tc.tile_pool(name="hs", bufs=1) as hpool,
            tc.tile_pool(name="sm", bufs=2) as smpool,
            tc.tile_pool(name="gsc", bufs=1) as gspool,
            tc.tile_pool(name="accp", bufs=2) as apool,
            tc.tile_pool(name="outp", bufs=2) as opool,
            tc.tile_pool(name="pmm", bufs=2, space="PSUM") as pmm,
            tc.tile_pool(name="peo", bufs=2, space="PSUM") as peo,
            tc.tile_pool(name="pmisc", bufs=2, space="PSUM") as pmisc,
        ):
            # --- prologue loads; embt piecewise so the first MMs start early ---
            embt_sb = cpool.tile([128, 2, KC, V], bf16)
            wg_sb = cpool.tile([128, 2, KC, E], bf16)
            for kc in range(KC):
                nc.sync.dma_start(embt_sb[:, 0, kc, :], embtd[0, kc])
            nc.sync.dma_start(wg_sb[:], wgd[:])
            iv_sb = cpool.tile([128, VC], f32)
            nc.sync.dma_start(iv_sb[:], ivd[:])
            ones_f16 = cpool.tile([1, 128], f16)
            nc.vector.memset(ones_f16[:], 1.0)
            ones128_bf = cpool.tile([1, 128], bf16)
            nc.vector.memset(ones128_bf[:], 1.0)
            ones4_bf = cpool.tile([E, 1], bf16)
            nc.vector.memset(ones4_bf[:], 1.0)
            bgr_sb = cpool.tile([1, E], bf16)
            nc.sync.dma_start(bgr_sb[:], bgrd[:])
            b1r_sb = cpool.tile([1, F], bf16)
            nc.sync.dma_start(b1r_sb[:], b1rd[:])
            x0_pre = []
            for t, xd in enumerate((x0d, x1d)):
                xs = xpool.tile([1, ST], f16, tag=f"x{t}")
                nc.sync.dma_start(xs[:], xd[0])
                x0_pre.append(xs)
            for kc in range(KC):
                nc.sync.dma_start(embt_sb[:, 1, kc, :], embtd[1, kc])
            b2_sb = cpool.tile([128, E], f32)
            nc.sync.dma_start(b2_sb[:], b2d[:])
            sel_sb = cpool.tile([E, E, 128], bf16)
            nc.sync.dma_start(sel_sb[:], seld[:])

            # --- phase 0a: exp'd gating tables (fills the first W1 DMA gap;
            #     the single Exp table-set load happens here, before any Silu) ---
            g_sb = cpool.tile([128, VC, 2, E], bf16)
            for t in range(2):
                for vc in range(VC):
                    psg = pmisc.tile([128, E], f32, tag="misc")
                    for kc in range(KC):
                        nc.tensor.matmul(
                            psg[:],
                            embt_sb[:, t, kc, vc * 128 : (vc + 1) * 128],
                            wg_sb[:, t, kc, :],
                            start=(kc == 0),
                            stop=(t == 1 and kc == KC - 1),
                        )
                    if t == 0:
                        # fold bg into table 0: psg += ones(v) x bg
                        nc.tensor.matmul(
                            psg[:], ones128_bf[:], bgr_sb[:],
                            start=False, stop=True,
                        )
                    nc.scalar.activation(g_sb[:, vc, t, :], psg[:], AF.Exp, bias=0.0)

            def build_masks(i, preloaded=None):
                """x-broadcast (K=1 matmul) + one-hot compares for the chunks
                present in supertile i."""
                pres = _present(i)
                ms = [{}, {}]
                for t, xd in enumerate((x0d, x1d)):
                    if preloaded is None:
                        xs = xpool.tile([1, ST], f16, tag=f"x{t}")
                        nc.sync.dma_start(xs[:], xd[i])
                    else:
                        xs = preloaded[t]
                    p = pmisc.tile([128, ST], f32, tag="misc")
                    nc.tensor.matmul(p[:], ones_f16[:], xs[:])
                    for vc in pres[t]:
                        m = mpool.tile([128, ST], bf16, tag=f"m{t}{vc}")
                        nc.vector.tensor_scalar(
                            m[:], p[:], iv_sb[:, vc : vc + 1], None, ALU.is_equal
                        )
                        ms[t][vc] = m
                return ms

            def emit_gating(i, masks):
                """Two one-hot gate selections (exp'd tables) multiplied on the
                DVE: expt = expG0[i0] * expG1[i1]."""
                pres = _present(i)
                sels = []
                for t in range(2):
                    sl = pmisc.tile([E, ST], f32, tag="misc")
                    for j, vc in enumerate(pres[t]):
                        nc.tensor.matmul(
                            sl[:],
                            g_sb[:, vc, t, :],
                            masks[t][vc][:],
                            start=(j == 0),
                            stop=(j == len(pres[t]) - 1),
                        )
                    sels.append(sl)
                # DVE may read only one PSUM operand per op: stage sel0 in SBUF
                s0 = smpool.tile([E, ST], f32, tag="s0")
                nc.vector.tensor_copy(s0[:], sels[0][:])
                expt = smpool.tile([E, ST], bf16, tag="expt")
                nc.vector.tensor_tensor(expt[:], s0[:], sels[1][:], ALU.mult)
                return expt

            cur_masks = build_masks(0, preloaded=x0_pre)
            cur_expt = emit_gating(0, cur_masks)

            # --- phase 0b: T tables (selection stationary layout, b1 folded) ---
            # last dims [8, 512]: piece fc//4, offset (fc%4)*128
            t_sb = cpool.tile([128, VC, 2, FC // 4, 512], bf16)
            for t in range(2):
                for fb in range(FB):
                    w1t = w1pool.tile([128, KC, 1024], bf16, tag="w1")
                    for kc in range(KC):
                        nc.sync.dma_start(w1t[:, kc, :], w1d[t, fb, kc])
                    for vc in range(VC):
                        ps = pmm.tile([128, 2, ST], f32, tag="mm")
                        for fp in range(FPB):
                            for kc in range(KC):
                                nc.tensor.matmul(
                                    ps[:, fp, :],
                                    embt_sb[:, t, kc, vc * 128 : (vc + 1) * 128],
                                    w1t[:, kc, fp * 512 : (fp + 1) * 512],
                                    start=(kc == 0),
                                    stop=(t == 1 and kc == KC - 1),
                                )
                            if t == 0:
                                f0 = fb * 1024 + fp * 512
                                nc.tensor.matmul(
                                    ps[:, fp, :], ones128_bf[:],
                                    b1r_sb[:, f0 : f0 + 512],
                                    start=False, stop=True,
                                )
                        # both 512-halves evacuated with one copy
                        nc.scalar.copy(
                            t_sb[:, vc, t, fb * FPB : (fb + 1) * FPB, :], ps[:]
                        )

            # --- remaining resident weights ---
            w2_sb = cpool.tile([128, E, KC, OUT], bf16)
            nc.sync.dma_start(w2_sb[:], w2d[:])

            for i in range(nst):
                pres = _present(i)
                chunks = [(t, vc) for t in range(2) for vc in pres[t]]
                expt = cur_expt

                def emit_recip_chain():
                    # sum-exp -> reciprocal -> broadcast to 128 rows (PSUM).
                    sp = pmisc.tile([1, ST], f32, tag="misc")
                    nc.tensor.matmul(sp[:], ones4_bf[:], expt[:])
                    rec = smpool.tile([1, ST], f32, tag="rec")
                    nc.vector.reciprocal(rec[:], sp[:])
                    recb = smpool.tile([1, ST], bf16, tag="recb")
                    nc.vector.tensor_copy(recb[:], rec[:])
                    rbp = pmisc.tile([128, ST], f32, tag="misc")
                    nc.tensor.matmul(rbp[:], ones128_bf[:], recb[:])
                    rbs = smpool.tile([128, ST], f32, tag="rbs")
                    nc.scalar.copy(rbs[:], rbp[:])
                    return rbs

                def emit_expert(e, acc):
                    # W2 for expert e (its 4 h pairs are ready) + gate-combine.
                    # gs evac on the scalar engine: the DVE FIFO must not gate
                    # the peo/pmisc PSUM rotation (head-of-line blocking).
                    eop = peo.tile([128, ST], f32, tag="eo")
                    for dc in range(KC):
                        fc = e * KC + dc
                        nc.tensor.matmul(
                            eop[:],
                            w2_sb[:, e, dc, :],
                            hs[fc // 2][:, fc % 2, :],
                            start=(dc == 0),
                            stop=(dc == KC - 1),
                        )
                    gp = pmisc.tile([128, ST], f32, tag="misc")
                    nc.tensor.matmul(gp[:], sel_sb[:, e, :], expt[:])
                    gs = gspool.tile([128, ST], f32, tag="gs")
                    nc.scalar.copy(gs[:], gp[:])
                    if e == 0:
                        nc.vector.scalar_tensor_tensor(
                            acc[:], eop[:], b2_sb[:, e : e + 1], gs[:],
                            ALU.add, ALU.mult,
                        )
                    else:
                        tmp = opool.tile([128, ST], f32, tag="outt")
                        nc.vector.scalar_tensor_tensor(
                            tmp[:], eop[:], b2_sb[:, e : e + 1], gs[:],
                            ALU.add, ALU.mult,
                        )
                        nc.vector.tensor_add(acc[:], acc[:], tmp[:])

                # --- selection + paired silu (b1 already inside T), with each
                # expert's W2+combine interleaved after its 4th silu pair ---
                next_masks = None
                hs = []
                acc = apool.tile([128, ST], f32, tag="acc")
                for pair in range(FC // 2):
                    if pair == 1:
                        rbs = emit_recip_chain()
                    if pair == 9 and i + 1 < nst:
                        next_masks = build_masks(i + 1)
                    hp = pmm.tile([128, 2, ST], f32, tag="mm")
                    for half in range(2):
                        fc = pair * 2 + half
                        for j, (t, vc) in enumerate(chunks):
                            nc.tensor.matmul(
                                hp[:, half, :],
                                t_sb[
                                    :, vc, t, fc // 4,
                                    (fc % 4) * 128 : (fc % 4 + 1) * 128,
                                ],
                                cur_masks[t][vc][:],
                                start=(j == 0),
                                stop=(j == len(chunks) - 1),
                            )
                    h_pair = hpool.tile([128, 2, ST], bf16, tag=f"hs{pair}")
                    nc.scalar.activation(h_pair[:], hp[:], AF.Silu, bias=0.0)
                    hs.append(h_pair)
                    if pair % 4 == 3:
                        emit_expert(pair // 4, acc)
                    if pair == 11 and i + 1 < nst:
                        cur_expt = emit_gating(i + 1, next_masks)

                outt = opool.tile([128, ST], f32, tag="outt")
                nc.vector.tensor_tensor(outt[:], acc[:], rbs[:], ALU.mult)
                nc.sync.dma_start(outd[:, i * ST : (i + 1) * ST], outt[:])
                if next_masks is not None:
                    cur_masks = next_masks

    if legalize:
        _legalize_waits(nc)
    mybir.codegen_inst_isa_subclasses(nc)
    return nc


def assign_slots(x):
    """Bucket tokens by (i0//128, i1//128) into 16 pure supertiles (512 slots,
    padded) + spill. Returns per-core slot->token maps and n_mixed."""
    x = np.asarray(x)
    slot_maps = []
    spills = []
    for c in range(NCORES):
        xc = x[c * BL : (c + 1) * BL]
        key = (xc[:, 0] // 128) * VC + xc[:, 1] // 128
        order = np.argsort(key, kind="stable")
        ks = key[order]
        slots = np.full(NPURE * ST, -1, dtype=np.int64)
        spill = []
        for b in range(NPURE):
            toks = order[ks == b]
            n = min(len(toks), ST)
            slots[b * ST : b * ST + n] = toks[:n]
            spill.extend(toks[ST:])
        slot_maps.append(slots)
        spills.append(np.array(spill, dtype=np.int64))
    n_mixed = max(
        (len(s) + ST - 1) // ST if len(s) else 0 for s in spills
    )
    full_maps = []
    for c in range(NCORES):
        m = np.full((NPURE + n_mixed) * ST, -1, dtype=np.int64)
        m[: NPURE * ST] = slot_maps[c]
        m[NPURE * ST : NPURE * ST + len(spills[c])] = spills[c]
        full_maps.append(m)
    return full_maps, n_mixed


def marshal_inputs(x, emb0, emb1, W1, b1, W2, b2, Wg, bg, slot_maps, n_mixed):
    """Host-side: cast/reshape full inputs into per-core in_maps."""
    nst = NPURE + n_mixed
    x = np.asarray(x)

    xh = {"x0": [], "x1": []}
    for c in range(NCORES):
        m = slot_maps[c]
        xc = x[c * BL : (c + 1) * BL]
        xv = np.zeros((len(m), 2), dtype=np.float16)
        valid = m >= 0
        xv[valid] = xc[m[valid]].astype(np.float16)
        # pad slots: -1 matches no iv entry -> zero one-hot -> output junk
        # that the host discards.
        xv[~valid] = -1.0
        xh["x0"].append(np.ascontiguousarray(xv[:, 0].reshape(nst, 1, ST)))
        xh["x1"].append(np.ascontiguousarray(xv[:, 1].reshape(nst, 1, ST)))

    shared = {}
    # embT[t, kc, p, v] = emb_t[v, kc*128 + p]
    embt = np.stack(
        [np.asarray(e).T.reshape(KC, 128, V) for e in (emb0, emb1)], axis=0
    )
    shared["embt"] = np.ascontiguousarray(embt.astype(BF16))
    # W1flat[k, f] with f = e*1024 + d
    w1flat = np.asarray(W1).transpose(1, 0, 2).reshape(IN, F)
    shared["w1m"] = np.ascontiguousarray(
        w1flat.reshape(2, KC, 128, FB, 1024).transpose(0, 3, 1, 2, 4).astype(BF16)
    )
    shared["b1row"] = np.ascontiguousarray(
        np.asarray(b1).reshape(1, F).astype(BF16)
    )
    shared["bgrow"] = np.ascontiguousarray(
        np.asarray(bg).reshape(1, E).astype(BF16)
    )
    shared["wgm"] = np.ascontiguousarray(
        np.asarray(Wg).reshape(2, KC, 128, E).transpose(2, 0, 1, 3).astype(BF16)
    )
    shared["w2s"] = np.ascontiguousarray(
        np.asarray(W2).reshape(E, KC, 128, OUT).transpose(2, 0, 1, 3).astype(BF16)
    )
    shared["b2s"] = np.ascontiguousarray(np.asarray(b2).T.astype(np.float32))
    shared["sels"] = np.ascontiguousarray(
        np.broadcast_to(np.eye(E, dtype=np.float32)[:, :, None], (E, E, 128)).astype(
            BF16
        )
    )
    shared["ivs"] = np.ascontiguousarray(
        (np.arange(VC)[None, :] * 128 + np.arange(128)[:, None]).astype(np.float32)
    )
    return [
        {**{k: v[c] for k, v in xh.items()}, **shared} for c in range(NCORES)
    ]


def kernel(x, emb0, emb1, W1, b1, W2, b2, Wg, bg):
    global LAST_EXEC_NS
    slot_maps, n_mixed = assign_slots(x)
    nc = build_program(n_mixed)
    in_maps = marshal_inputs(
        x, emb0, emb1, W1, b1, W2, b2, Wg, bg, slot_maps, n_mixed
    )
    trace = os.environ.get("BASSMOE_TRACE", "0") == "1"
    res = run_bass_kernel_spmd(nc, in_maps, list(range(NCORES)), trace=trace)
    LAST_EXEC_NS = res.exec_time_ns
    out = np.empty((B, OUT), dtype=np.float32)
    for c in range(NCORES):
        m = slot_maps[c]
        valid = m >= 0
        r = res.results[c]["out"]  # [128, nst*ST]
        out[c * BL + m[valid], :] = r[:, valid].T
    return out


# revision 3
# speedup vs baseline: 1.0685x; 1.0565x over previous
"""MoE model via vocab-precompute + bucketed one-hot selection on 8 TRN2 cores.

v2 replaced the dense W1 matmul with a per-vocab precompute (T tables) and a
per-token one-hot selection (K=512 per table).  v3 bucketed tokens by their
(i0//128, i1//128) vocab-chunk pair into 16 "pure" supertiles + n_mixed spill
supertiles, cutting selection to one matmul per table per feature chunk.

v4 removes the scalar-engine pacing found in the v3 trace:
  - b1 and bg are folded into the T/G precompute via an extra K=1 matmul row
    (ones stationary, bias as moving), so the silu evac needs no bias AP.
  - silu evacs are PAIRED: pmm tiles span 2 PSUM banks [128, 2, 512]; one
    ACTIVATE covers 1024 columns, amortizing the 352-cycle fixed cost.
  - exp FACTORIZES over the two tables: exp(G0[i0]+G1[i1]+bg) =
    expG0[i0] * expG1[i1].  exp moves into the precompute (G tables stored
    exp'd); the main loop multiplies two one-hot gate selections on the DVE.
    No Exp in the loop -> no ACT table-set switches at all.
  - the combine's gate operand is read straight from its broadcast PSUM.
"""

import os
import numpy as np
import ml_dtypes

import concourse.bass as bass
import concourse.mybir as mybir
import concourse.tile as tile
from concourse.bass_utils import run_bass_kernel_spmd

BF16 = ml_dtypes.bfloat16

B = 65536
V = 512
D = 1024
IN = 2048
E = 4
OUT = 128
NCORES = 8
BL = B // NCORES          # tokens per core
ST = 512                  # tokens per supertile (max PSUM free dim, fp32)
F = E * D                 # 4096 selected features (expert-major)
FC = F // 128             # 32 feature chunks
VC = V // 128             # 4 vocab chunks
KC = D // 128             # 8 contraction chunks per table (precompute)
FB = 4                    # W1 f-blocks streamed (1024 feats each)
FPB = 2                   # 512-wide f-pieces per f-block
NPURE = VC * VC           # 16 pure supertiles (one per chunk pair)

LAST_EXEC_NS = None       # set when BASSMOE_TRACE=1


def _legalize_waits(nc, max_waits=1):
    """This walrus build rejects instructions carrying more than ~1 sync-wait
    command; hoist all but the last wait onto single-wait NoOps."""
    for f in nc.m.functions:
        for bb in f.blocks:
            insts = bb.instructions
            if not any(
                inst.sync_info is not None and len(inst.sync_info.on_wait) > max_waits
                for inst in insts
            ):
                continue
            new = []
            for inst in insts:
                si = inst.sync_info
                waits = list(si.on_wait) if si is not None else []
                if len(waits) > max_waits:
                    for w in waits[:-max_waits]:
                        nop = mybir.InstNoOp(
                            name=f"legw-{nc.next_id()}", ins=[], outs=[]
                        )
                        nop.engine = inst.engine
                        nop.sync_info = mybir.SyncInfo(on_wait=[w], on_update=[])
                        new.append(nop)
                    inst.sync_info = mybir.SyncInfo(
                        on_wait=waits[-max_waits:], on_update=list(si.on_update)
                    )
                new.append(inst)
            bb.instructions = new


def _present(s):
    """Vocab chunks present in supertile s, per table."""
    if s < NPURE:
        return [[s // VC], [s % VC]]
    return [list(range(VC)), list(range(VC))]


def build_program(n_mixed, legalize=True):
    nst = NPURE + n_mixed
    dt = mybir.dt
    f32, bf16, f16 = dt.float32, dt.bfloat16, dt.float16
    AF = mybir.ActivationFunctionType
    ALU = mybir.AluOpType

    nc = bass.Bass()

    x0d = nc.dram_tensor("x0", [nst, 1, ST], f16, kind="ExternalInput")
    x1d = nc.dram_tensor("x1", [nst, 1, ST], f16, kind="ExternalInput")
    # emb pre-transposed: embT[t, kc, p, v] = emb_t[v, kc*128+p]
    embtd = nc.dram_tensor("embt", [2, KC, 128, V], bf16, kind="ExternalInput")
    # W1 re-laid: w1m[t, fb, kc, p, ff] = W1flat[t*1024+kc*128+p, fb*1024+ff]
    w1d = nc.dram_tensor("w1m", [2, FB, KC, 128, 1024], bf16, kind="ExternalInput")
    b1rd = nc.dram_tensor("b1row", [1, F], bf16, kind="ExternalInput")
    bgrd = nc.dram_tensor("bgrow", [1, E], bf16, kind="ExternalInput")
    wgd = nc.dram_tensor("wgm", [128, 2, KC, E], bf16, kind="ExternalInput")
    w2d = nc.dram_tensor("w2s", [128, E, KC, OUT], bf16, kind="ExternalInput")
    b2d = nc.dram_tensor("b2s", [128, E], f32, kind="ExternalInput")
    seld = nc.dram_tensor("sels", [E, E, 128], bf16, kind="ExternalInput")
    ivd = nc.dram_tensor("ivs", [128, VC], f32, kind="ExternalInput")
    outd = nc.dram_tensor("out", [128, nst * ST], f32, kind="ExternalOutput")

    with tile.TileContext(nc) as tc:
        with (
            tc.tile_pool(name="const", bufs=1) as cpool,
            tc.tile_pool(name="w1st", bufs=2) as w1pool,
            tc.tile_pool(name="xt", bufs=2) as xpool,
            tc.tile_pool(name="mask", bufs=2) as mpool,
            tc.tile_pool(name="hs", bufs=1) as hpool,
            tc.tile_pool(name="sm", bufs=2) as smpool,
            tc.tile_pool(name="gsc", bufs=1) as gspool,
            tc.tile_pool(name="accp", bufs=2) as apool,
            tc.tile_pool(name="outp", bufs=2) as opool,
            tc.tile_pool(name="pmm", bufs=2, space="PSUM") as pmm,
            tc.tile_pool(name="peo", bufs=2, space="PSUM") as peo,
            tc.tile_pool(name="pmisc", bufs=2, space="PSUM") as pmisc,
        ):
            # --- prologue loads; embt piecewise so the first MMs start early ---
            # first compute needs embt-t0 + the first W1 block: issue those
            # DMAs first, interleaved per kc, so T matmuls start at ~3us; all
            # small constants (with their ~1us fixed DMA costs) queue after.
            embt_sb = cpool.tile([128, 2, KC, V], bf16)
            wg_sb = cpool.tile([128, 2, KC, E], bf16)
            w1t00 = w1pool.tile([128, KC, 1024], bf16, tag="w1")
            b1r_sb = cpool.tile([1, F], bf16)
            for kc in range(KC):
                nc.sync.dma_start(embt_sb[:, 0, kc, :], embtd[0, kc])
                nc.sync.dma_start(w1t00[:, kc, :], w1d[0, 0, kc])
            nc.sync.dma_start(b1r_sb[:], b1rd[:])
            nc.sync.dma_start(wg_sb[:], wgd[:])
            iv_sb = cpool.tile([128, VC], f32)
            nc.sync.dma_start(iv_sb[:], ivd[:])
            ones_f16 = cpool.tile([1, 128], f16)
            nc.vector.memset(ones_f16[:], 1.0)
            ones128_bf = cpool.tile([1, 128], bf16)
            nc.vector.memset(ones128_bf[:], 1.0)
            ones4_bf = cpool.tile([E, 1], bf16)
            nc.vector.memset(ones4_bf[:], 1.0)
            bgr_sb = cpool.tile([1, E], bf16)
            nc.sync.dma_start(bgr_sb[:], bgrd[:])
            x0_pre = []
            for t, xd in enumerate((x0d, x1d)):
                xs = xpool.tile([1, ST], f16, tag=f"x{t}")
                nc.sync.dma_start(xs[:], xd[0])
                x0_pre.append(xs)
            t_sb = cpool.tile([128, VC, 2, FC // 4, 512], bf16)

            def emit_t_block(t, fb, w1t):
                for vc in range(VC):
                    ps = pmm.tile([128, 2, ST], f32, tag="mm")
                    for fp in range(FPB):
                        for kc in range(KC):
                            nc.tensor.matmul(
                                ps[:, fp, :],
                                embt_sb[:, t, kc, vc * 128 : (vc + 1) * 128],
                                w1t[:, kc, fp * 512 : (fp + 1) * 512],
                                start=(kc == 0),
                                stop=(t == 1 and kc == KC - 1),
                            )
                        if t == 0:
                            f0 = fb * 1024 + fp * 512
                            nc.tensor.matmul(
                                ps[:, fp, :], ones128_bf[:],
                                b1r_sb[:, f0 : f0 + 512],
                                start=False, stop=True,
                            )
                    nc.scalar.copy(
                        t_sb[:, vc, t, fb * FPB : (fb + 1) * FPB, :], ps[:]
                    )

            # block (0,0) computes while the rest of the inputs stream in
            emit_t_block(0, 0, w1t00)
            for kc in range(KC):
                nc.sync.dma_start(embt_sb[:, 1, kc, :], embtd[1, kc])
            b2_sb = cpool.tile([128, E], f32)
            nc.sync.dma_start(b2_sb[:], b2d[:])
            sel_sb = cpool.tile([E, E, 128], bf16)
            nc.sync.dma_start(sel_sb[:], seld[:])

            # --- phase 0a: exp'd gating tables (the single Exp table-set load
            #     happens here, before any Silu) ---
            g_sb = cpool.tile([128, VC, 2, E], bf16)
            for t in range(2):
                for vc in range(VC):
                    psg = pmisc.tile([128, E], f32, tag="misc")
                    for kc in range(KC):
                        nc.tensor.matmul(
                            psg[:],
                            embt_sb[:, t, kc, vc * 128 : (vc + 1) * 128],
                            wg_sb[:, t, kc, :],
                            start=(kc == 0),
                            stop=(t == 1 and kc == KC - 1),
                        )
                    if t == 0:
                        # fold bg into table 0: psg += ones(v) x bg
                        nc.tensor.matmul(
                            psg[:], ones128_bf[:], bgr_sb[:],
                            start=False, stop=True,
                        )
                    nc.scalar.activation(g_sb[:, vc, t, :], psg[:], AF.Exp, bias=0.0)

            def build_masks(i, preloaded=None):
                """x-broadcast (K=1 matmul) + one-hot compares for the chunks
                present in supertile i."""
                pres = _present(i)
                ms = [{}, {}]
                for t, xd in enumerate((x0d, x1d)):
                    if preloaded is None:
                        xs = xpool.tile([1, ST], f16, tag=f"x{t}")
                        nc.sync.dma_start(xs[:], xd[i])
                    else:
                        xs = preloaded[t]
                    p = pmisc.tile([128, ST], f32, tag="misc")
                    nc.tensor.matmul(p[:], ones_f16[:], xs[:])
                    for vc in pres[t]:
                        m = mpool.tile([128, ST], bf16, tag=f"m{t}{vc}")
                        nc.vector.tensor_scalar(
                            m[:], p[:], iv_sb[:, vc : vc + 1], None, ALU.is_equal
                        )
                        ms[t][vc] = m
                return ms

            def emit_gating(i, masks):
                """Two one-hot gate selections (exp'd tables) multiplied on the
                DVE: expt = expG0[i0] * expG1[i1]."""
                pres = _present(i)
                sels = []
                for t in range(2):
                    sl = pmisc.tile([E, ST], f32, tag="misc")
                    for j, vc in enumerate(pres[t]):
                        nc.tensor.matmul(
                            sl[:],
                            g_sb[:, vc, t, :],
                            masks[t][vc][:],
                            start=(j == 0),
                            stop=(j == len(pres[t]) - 1),
                        )
                    sels.append(sl)
                # DVE may read only one PSUM operand per op: stage sel0 in SBUF
                s0 = smpool.tile([E, ST], f32, tag="s0")
                nc.vector.tensor_copy(s0[:], sels[0][:])
                expt = smpool.tile([E, ST], bf16, tag="expt")
                nc.vector.tensor_tensor(expt[:], s0[:], sels[1][:], ALU.mult)
                return expt

            cur_masks = build_masks(0, preloaded=x0_pre)
            cur_expt = emit_gating(0, cur_masks)

            # --- phase 0b: remaining T blocks (block (0,0) ran above) ---
            for t in range(2):
                for fb in range(FB):
                    if t == 0 and fb == 0:
                        continue
                    w1t = w1pool.tile([128, KC, 1024], bf16, tag="w1")
                    for kc in range(KC):
                        nc.sync.dma_start(w1t[:, kc, :], w1d[t, fb, kc])
                    emit_t_block(t, fb, w1t)

            # --- remaining resident weights ---
            w2_sb = cpool.tile([128, E, KC, OUT], bf16)
            nc.sync.dma_start(w2_sb[:], w2d[:])

            for i in range(nst):
                pres = _present(i)
                chunks = [(t, vc) for t in range(2) for vc in pres[t]]
                expt = cur_expt

                def emit_recip_chain():
                    # sum-exp -> reciprocal -> broadcast to 128 rows (PSUM).
                    sp = pmisc.tile([1, ST], f32, tag="misc")
                    nc.tensor.matmul(sp[:], ones4_bf[:], expt[:])
                    rec = smpool.tile([1, ST], f32, tag="rec")
                    nc.vector.reciprocal(rec[:], sp[:])
                    recb = smpool.tile([1, ST], bf16, tag="recb")
                    nc.vector.tensor_copy(recb[:], rec[:])
                    rbp = pmisc.tile([128, ST], f32, tag="misc")
                    nc.tensor.matmul(rbp[:], ones128_bf[:], recb[:])
                    rbs = smpool.tile([128, ST], f32, tag="rbs")
                    nc.scalar.copy(rbs[:], rbp[:])
                    return rbs

                def emit_expert(e, acc):
                    # W2 for expert e (its 4 h pairs are ready) + gate-combine.
                    # gs evac on the scalar engine: the DVE FIFO must not gate
                    # the peo/pmisc PSUM rotation (head-of-line blocking).
                    eop = peo.tile([128, ST], f32, tag="eo")
                    for dc in range(KC):
                        fc = e * KC + dc
                        nc.tensor.matmul(
                            eop[:],
                            w2_sb[:, e, dc, :],
                            hs[fc // 2][:, fc % 2, :],
                            start=(dc == 0),
                            stop=(dc == KC - 1),
                        )
                    gp = pmisc.tile([128, ST], f32, tag="misc")
                    nc.tensor.matmul(gp[:], sel_sb[:, e, :], expt[:])
                    gs = gspool.tile([128, ST], f32, tag="gs")
                    nc.scalar.copy(gs[:], gp[:])
                    if e == 0:
                        nc.vector.scalar_tensor_tensor(
                            acc[:], eop[:], b2_sb[:, e : e + 1], gs[:],
                            ALU.add, ALU.mult,
                        )
                    else:
                        tmp = opool.tile([128, ST], f32, tag="outt")
                        nc.vector.scalar_tensor_tensor(
                            tmp[:], eop[:], b2_sb[:, e : e + 1], gs[:],
                            ALU.add, ALU.mult,
                        )
                        nc.vector.tensor_add(acc[:], acc[:], tmp[:])

                # --- selection + paired silu (b1 already inside T), with each
                # expert's W2+combine interleaved after its 4th silu pair ---
                next_masks = None
                hs = []
                acc = apool.tile([128, ST], f32, tag="acc")
                for pair in range(FC // 2):
                    if pair == 1:
                        rbs = emit_recip_chain()
                    if pair == 9 and i + 1 < nst:
                        next_masks = build_masks(i + 1)
                    hp = pmm.tile([128, 2, ST], f32, tag="mm")
                    for half in range(2):
                        fc = pair * 2 + half
                        for j, (t, vc) in enumerate(chunks):
                            nc.tensor.matmul(
                                hp[:, half, :],
                                t_sb[
                                    :, vc, t, fc // 4,
                                    (fc % 4) * 128 : (fc % 4 + 1) * 128,
                                ],
                                cur_masks[t][vc][:],
                                start=(j == 0),
                                stop=(j == len(chunks) - 1),
                            )
                    h_pair = hpool.tile([128, 2, ST], bf16, tag=f"hs{pair}")
                    nc.scalar.activation(h_pair[:], hp[:], AF.Silu, bias=0.0)
                    hs.append(h_pair)
                    if pair % 4 == 3:
                        emit_expert(pair // 4, acc)
                    if pair == 11 and i + 1 < nst:
                        cur_expt = emit_gating(i + 1, next_masks)

                outt = opool.tile([128, ST], f32, tag="outt")
                nc.vector.tensor_tensor(outt[:], acc[:], rbs[:], ALU.mult)
                nc.sync.dma_start(outd[:, i * ST : (i + 1) * ST], outt[:])
                if next_masks is not None:
                    cur_masks = next_masks

    if legalize:
        _legalize_waits(nc)
    mybir.codegen_inst_isa_subclasses(nc)
    return nc


def assign_slots(x):
    """Bucket tokens by (i0//128, i1//128) into 16 pure supertiles (512 slots,
    padded) + spill. Returns per-core slot->token maps and n_mixed."""
    x = np.asarray(x)
    slot_maps = []
    spills = []
    for c in range(NCORES):
        xc = x[c * BL : (c + 1) * BL]
        key = (xc[:, 0] // 128) * VC + xc[:, 1] // 128
        order = np.argsort(key, kind="stable")
        ks = key[order]
        slots = np.full(NPURE * ST, -1, dtype=np.int64)
        spill = []
        for b in range(NPURE):
            toks = order[ks == b]
            n = min(len(toks), ST)
            slots[b * ST : b * ST + n] = toks[:n]
            spill.extend(toks[ST:])
        slot_maps.append(slots)
        spills.append(np.array(spill, dtype=np.int64))
    n_mixed = max(
        (len(s) + ST - 1) // ST if len(s) else 0 for s in spills
    )
    full_maps = []
    for c in range(NCORES):
        m = np.full((NPURE + n_mixed) * ST, -1, dtype=np.int64)
        m[: NPURE * ST] = slot_maps[c]
        m[NPURE * ST : NPURE * ST + len(spills[c])] = spills[c]
        full_maps.append(m)
    return full_maps, n_mixed


def marshal_inputs(x, emb0, emb1, W1, b1, W2, b2, Wg, bg, slot_maps, n_mixed):
    """Host-side: cast/reshape full inputs into per-core in_maps."""
    nst = NPURE + n_mixed
    x = np.asarray(x)

    xh = {"x0": [], "x1": []}
    for c in range(NCORES):
        m = slot_maps[c]
        xc = x[c * BL : (c + 1) * BL]
        xv = np.zeros((len(m), 2), dtype=np.float16)
        valid = m >= 0
        xv[valid] = xc[m[valid]].astype(np.float16)
        # pad slots: -1 matches no iv entry -> zero one-hot -> output junk
        # that the host discards.
        xv[~valid] = -1.0
        xh["x0"].append(np.ascontiguousarray(xv[:, 0].reshape(nst, 1, ST)))
        xh["x1"].append(np.ascontiguousarray(xv[:, 1].reshape(nst, 1, ST)))

    shared = {}
    # embT[t, kc, p, v] = emb_t[v, kc*128 + p]
    embt = np.stack(
        [np.asarray(e).T.reshape(KC, 128, V) for e in (emb0, emb1)], axis=0
    )
    shared["embt"] = np.ascontiguousarray(embt.astype(BF16))
    # W1flat[k, f] with f = e*1024 + d
    w1flat = np.asarray(W1).transpose(1, 0, 2).reshape(IN, F)
    shared["w1m"] = np.ascontiguousarray(
        w1flat.reshape(2, KC, 128, FB, 1024).transpose(0, 3, 1, 2, 4).astype(BF16)
    )
    shared["b1row"] = np.ascontiguousarray(
        np.asarray(b1).reshape(1, F).astype(BF16)
    )
    shared["bgrow"] = np.ascontiguousarray(
        np.asarray(bg).reshape(1, E).astype(BF16)
    )
    shared["wgm"] = np.ascontiguousarray(
        np.asarray(Wg).reshape(2, KC, 128, E).transpose(2, 0, 1, 3).astype(BF16)
    )
    shared["w2s"] = np.ascontiguousarray(
        np.asarray(W2).reshape(E, KC, 128, OUT).transpose(2, 0, 1, 3).astype(BF16)
    )
    shared["b2s"] = np.ascontiguousarray(np.asarray(b2).T.astype(np.float32))
    shared["sels"] = np.ascontiguousarray(
        np.broadcast_to(np.eye(E, dtype=np.float32)[:, :, None], (E, E, 128)).astype(
            BF16
        )
    )
    shared["ivs"] = np.ascontiguousarray(
        (np.arange(VC)[None, :] * 128 + np.arange(128)[:, None]).astype(np.float32)
    )
    return [
        {**{k: v[c] for k, v in xh.items()}, **shared} for c in range(NCORES)
    ]


def kernel(x, emb0, emb1, W1, b1, W2, b2, Wg, bg):
    global LAST_EXEC_NS
    slot_maps, n_mixed = assign_slots(x)
    nc = build_program(n_mixed)
    in_maps = marshal_inputs(
        x, emb0, emb1, W1, b1, W2, b2, Wg, bg, slot_maps, n_mixed
    )
    trace = os.environ.get("BASSMOE_TRACE", "0") == "1"
    res = run_bass_kernel_spmd(nc, in_maps, list(range(NCORES)), trace=trace)
    LAST_EXEC_NS = res.exec_time_ns
    out = np.empty((B, OUT), dtype=np.float32)
    for c in range(NCORES):
        m = slot_maps[c]
        valid = m >= 0
        r = res.results[c]["out"]  # [128, nst*ST]
        out[c * BL + m[valid], :] = r[:, valid].T
    return out


# revision 4
# speedup vs baseline: 1.0720x; 1.0032x over previous
"""MoE model via vocab-precompute + bucketed one-hot selection on 8 TRN2 cores.

v2 replaced the dense W1 matmul with a per-vocab precompute (T tables) and a
per-token one-hot selection (K=512 per table).  v3 bucketed tokens by their
(i0//128, i1//128) vocab-chunk pair into 16 "pure" supertiles + n_mixed spill
supertiles, cutting selection to one matmul per table per feature chunk.

v4 removes the scalar-engine pacing found in the v3 trace:
  - b1 and bg are folded into the T/G precompute via an extra K=1 matmul row
    (ones stationary, bias as moving), so the silu evac needs no bias AP.
  - silu evacs are PAIRED: pmm tiles span 2 PSUM banks [128, 2, 512]; one
    ACTIVATE covers 1024 columns, amortizing the 352-cycle fixed cost.
  - exp FACTORIZES over the two tables: exp(G0[i0]+G1[i1]+bg) =
    expG0[i0] * expG1[i1].  exp moves into the precompute (G tables stored
    exp'd); the main loop multiplies two one-hot gate selections on the DVE.
    No Exp in the loop -> no ACT table-set switches at all.
  - the combine's gate operand is read straight from its broadcast PSUM.
"""

import os
import numpy as np
import ml_dtypes

import concourse.bass as bass
import concourse.mybir as mybir
import concourse.tile as tile
from concourse.bass_utils import run_bass_kernel_spmd

BF16 = ml_dtypes.bfloat16

B = 65536
V = 512
D = 1024
IN = 2048
E = 4
OUT = 128
NCORES = 8
BL = B // NCORES          # tokens per core
ST = 512                  # tokens per supertile (max PSUM free dim, fp32)
F = E * D                 # 4096 selected features (expert-major)
FC = F // 128             # 32 feature chunks
VC = V // 128             # 4 vocab chunks
KC = D // 128             # 8 contraction chunks per table (precompute)
FB = 4                    # W1 f-blocks streamed (1024 feats each)
FPB = 2                   # 512-wide f-pieces per f-block
NPURE = VC * VC           # 16 pure supertiles (one per chunk pair)
STM = 256                 # spill-supertile width (few tokens, cheaper MMs)

LAST_EXEC_NS = None       # set when BASSMOE_TRACE=1


def _legalize_waits(nc, max_waits=1):
    """This walrus build rejects instructions carrying more than ~1 sync-wait
    command; hoist all but the last wait onto single-wait NoOps."""
    for f in nc.m.functions:
        for bb in f.blocks:
            insts = bb.instructions
            if not any(
                inst.sync_info is not None and len(inst.sync_info.on_wait) > max_waits
                for inst in insts
            ):
                continue
            new = []
            for inst in insts:
                si = inst.sync_info
                waits = list(si.on_wait) if si is not None else []
                if len(waits) > max_waits:
                    for w in waits[:-max_waits]:
                        nop = mybir.InstNoOp(
                            name=f"legw-{nc.next_id()}", ins=[], outs=[]
                        )
                        nop.engine = inst.engine
                        nop.sync_info = mybir.SyncInfo(on_wait=[w], on_update=[])
                        new.append(nop)
                    inst.sync_info = mybir.SyncInfo(
                        on_wait=waits[-max_waits:], on_update=list(si.on_update)
                    )
                new.append(inst)
            bb.instructions = new


def _width(s):
    return ST if s < NPURE else STM


def _off(s):
    return s * ST if s <= NPURE else NPURE * ST + (s - NPURE) * STM


def _present(s):
    """Vocab chunks present in supertile s, per table."""
    if s < NPURE:
        return [[s // VC], [s % VC]]
    return [list(range(VC)), list(range(VC))]


def build_program(n_mixed, legalize=True):
    nst = NPURE + n_mixed
    dt = mybir.dt
    f32, bf16, f16 = dt.float32, dt.bfloat16, dt.float16
    AF = mybir.ActivationFunctionType
    ALU = mybir.AluOpType

    nc = bass.Bass()

    tot = NPURE * ST + n_mixed * STM
    x0d = nc.dram_tensor("x0", [1, tot], f16, kind="ExternalInput")
    x1d = nc.dram_tensor("x1", [1, tot], f16, kind="ExternalInput")
    # emb pre-transposed: embT[t, kc, p, v] = emb_t[v, kc*128+p]
    embtd = nc.dram_tensor("embt", [2, KC, 128, V], bf16, kind="ExternalInput")
    # W1 re-laid: w1m[t, fb, kc, p, ff] = W1flat[t*1024+kc*128+p, fb*1024+ff]
    w1d = nc.dram_tensor("w1m", [2, FB, KC, 128, 1024], bf16, kind="ExternalInput")
    b1rd = nc.dram_tensor("b1row", [1, F], bf16, kind="ExternalInput")
    bgrd = nc.dram_tensor("bgrow", [1, E], bf16, kind="ExternalInput")
    wgd = nc.dram_tensor("wgm", [128, 2, KC, E], bf16, kind="ExternalInput")
    w2d = nc.dram_tensor("w2s", [128, E, KC, OUT], bf16, kind="ExternalInput")
    b2d = nc.dram_tensor("b2s", [128, E], f32, kind="ExternalInput")
    seld = nc.dram_tensor("sels", [E, E, 128], bf16, kind="ExternalInput")
    ivd = nc.dram_tensor("ivs", [128, VC], f32, kind="ExternalInput")
    outd = nc.dram_tensor("out", [128, tot], f32, kind="ExternalOutput")

    with tile.TileContext(nc) as tc:
        with (
            tc.tile_pool(name="const", bufs=1) as cpool,
            tc.tile_pool(name="w1st", bufs=2) as w1pool,
            tc.tile_pool(name="xt", bufs=2) as xpool,
            tc.tile_pool(name="mask", bufs=2) as mpool,
            tc.tile_pool(name="hs", bufs=1) as hpool,
            tc.tile_pool(name="sm", bufs=2) as smpool,
            tc.tile_pool(name="gsc", bufs=1) as gspool,
            tc.tile_pool(name="accp", bufs=2) as apool,
            tc.tile_pool(name="outp", bufs=2) as opool,
            tc.tile_pool(name="pmm", bufs=2, space="PSUM") as pmm,
            tc.tile_pool(name="peo", bufs=2, space="PSUM") as peo,
            tc.tile_pool(name="pmisc", bufs=2, space="PSUM") as pmisc,
        ):
            # --- prologue loads; embt piecewise so the first MMs start early ---
            # first compute needs embt-t0 + the first W1 block: issue those
            # DMAs first, interleaved per kc, so T matmuls start at ~3us; all
            # small constants (with their ~1us fixed DMA costs) queue after.
            embt_sb = cpool.tile([128, 2, KC, V], bf16)
            wg_sb = cpool.tile([128, 2, KC, E], bf16)
            w1t00 = w1pool.tile([128, KC, 1024], bf16, tag="w1")
            b1r_sb = cpool.tile([1, F], bf16)
            for kc in range(KC):
                nc.sync.dma_start(embt_sb[:, 0, kc, :], embtd[0, kc])
                nc.sync.dma_start(w1t00[:, kc, :], w1d[0, 0, kc])
            nc.sync.dma_start(b1r_sb[:], b1rd[:])
            nc.sync.dma_start(wg_sb[:], wgd[:])
            iv_sb = cpool.tile([128, VC], f32)
            nc.sync.dma_start(iv_sb[:], ivd[:])
            ones_f16 = cpool.tile([1, 128], f16)
            nc.vector.memset(ones_f16[:], 1.0)
            ones128_bf = cpool.tile([1, 128], bf16)
            nc.vector.memset(ones128_bf[:], 1.0)
            ones4_bf = cpool.tile([E, 1], bf16)
            nc.vector.memset(ones4_bf[:], 1.0)
            bgr_sb = cpool.tile([1, E], bf16)
            nc.sync.dma_start(bgr_sb[:], bgrd[:])
            x0_pre = []
            for t, xd in enumerate((x0d, x1d)):
                xs = xpool.tile([1, ST], f16, tag=f"x{t}")
                nc.sync.dma_start(xs[:], xd[:, 0:ST])
                x0_pre.append(xs)
            t_sb = cpool.tile([128, VC, 2, FC // 4, 512], bf16)

            def emit_t_block(t, fb, w1t):
                for vc in range(VC):
                    ps = pmm.tile([128, 2, ST], f32, tag="mm")
                    for fp in range(FPB):
                        for kc in range(KC):
                            nc.tensor.matmul(
                                ps[:, fp, :],
                                embt_sb[:, t, kc, vc * 128 : (vc + 1) * 128],
                                w1t[:, kc, fp * 512 : (fp + 1) * 512],
                                start=(kc == 0),
                                stop=(t == 1 and kc == KC - 1),
                            )
                        if t == 0:
                            f0 = fb * 1024 + fp * 512
                            nc.tensor.matmul(
                                ps[:, fp, :], ones128_bf[:],
                                b1r_sb[:, f0 : f0 + 512],
                                start=False, stop=True,
                            )
                    nc.scalar.copy(
                        t_sb[:, vc, t, fb * FPB : (fb + 1) * FPB, :], ps[:]
                    )

            # block (0,0) computes while the rest of the inputs stream in
            emit_t_block(0, 0, w1t00)
            for kc in range(KC):
                nc.sync.dma_start(embt_sb[:, 1, kc, :], embtd[1, kc])
            b2_sb = cpool.tile([128, E], f32)
            nc.sync.dma_start(b2_sb[:], b2d[:])
            sel_sb = cpool.tile([E, E, 128], bf16)
            nc.sync.dma_start(sel_sb[:], seld[:])

            # --- phase 0a: exp'd gating tables (the single Exp table-set load
            #     happens here, before any Silu) ---
            g_sb = cpool.tile([128, VC, 2, E], bf16)
            for t in range(2):
                for vc in range(VC):
                    psg = pmisc.tile([128, E], f32, tag="misc")
                    for kc in range(KC):
                        nc.tensor.matmul(
                            psg[:],
                            embt_sb[:, t, kc, vc * 128 : (vc + 1) * 128],
                            wg_sb[:, t, kc, :],
                            start=(kc == 0),
                            stop=(t == 1 and kc == KC - 1),
                        )
                    if t == 0:
                        # fold bg into table 0: psg += ones(v) x bg
                        nc.tensor.matmul(
                            psg[:], ones128_bf[:], bgr_sb[:],
                            start=False, stop=True,
                        )
                    nc.scalar.activation(g_sb[:, vc, t, :], psg[:], AF.Exp, bias=0.0)

            def build_masks(i, preloaded=None):
                """x-broadcast (K=1 matmul) + one-hot compares for the chunks
                present in supertile i."""
                pres = _present(i)
                w = _width(i)
                ms = [{}, {}]
                for t, xd in enumerate((x0d, x1d)):
                    if preloaded is None:
                        xs = xpool.tile([1, w], f16, tag=f"x{t}")
                        nc.sync.dma_start(xs[:], xd[:, _off(i) : _off(i) + w])
                    else:
                        xs = preloaded[t]
                    p = pmisc.tile([128, w], f32, tag="misc")
                    nc.tensor.matmul(p[:], ones_f16[:], xs[:])
                    for vc in pres[t]:
                        m = mpool.tile([128, w], bf16, tag=f"m{t}{vc}")
                        nc.vector.tensor_scalar(
                            m[:], p[:], iv_sb[:, vc : vc + 1], None, ALU.is_equal
                        )
                        ms[t][vc] = m
                return ms

            def emit_gating(i, masks):
                """Two one-hot gate selections (exp'd tables) multiplied on the
                DVE: expt = expG0[i0] * expG1[i1]."""
                pres = _present(i)
                w = _width(i)
                sels = []
                for t in range(2):
                    sl = pmisc.tile([E, w], f32, tag="misc")
                    for j, vc in enumerate(pres[t]):
                        nc.tensor.matmul(
                            sl[:],
                            g_sb[:, vc, t, :],
                            masks[t][vc][:],
                            start=(j == 0),
                            stop=(j == len(pres[t]) - 1),
                        )
                    sels.append(sl)
                # DVE may read only one PSUM operand per op: stage sel0 in SBUF
                s0 = smpool.tile([E, w], f32, tag="s0")
                nc.vector.tensor_copy(s0[:], sels[0][:])
                expt = smpool.tile([E, w], bf16, tag="expt")
                nc.vector.tensor_tensor(expt[:], s0[:], sels[1][:], ALU.mult)
                return expt

            cur_masks = build_masks(0, preloaded=x0_pre)
            cur_expt = emit_gating(0, cur_masks)

            # --- phase 0b: remaining T blocks (block (0,0) ran above) ---
            for t in range(2):
                for fb in range(FB):
                    if t == 0 and fb == 0:
                        continue
                    w1t = w1pool.tile([128, KC, 1024], bf16, tag="w1")
                    for kc in range(KC):
                        nc.sync.dma_start(w1t[:, kc, :], w1d[t, fb, kc])
                    emit_t_block(t, fb, w1t)

            # --- remaining resident weights ---
            w2_sb = cpool.tile([128, E, KC, OUT], bf16)
            nc.sync.dma_start(w2_sb[:], w2d[:])

            for i in range(nst):
                pres = _present(i)
                w = _width(i)
                chunks = [(t, vc) for t in range(2) for vc in pres[t]]
                expt = cur_expt

                def emit_recip_chain():
                    # sum-exp -> reciprocal -> broadcast to 128 rows (PSUM).
                    sp = pmisc.tile([1, w], f32, tag="misc")
                    nc.tensor.matmul(sp[:], ones4_bf[:], expt[:])
                    rec = smpool.tile([1, w], f32, tag="rec")
                    nc.vector.reciprocal(rec[:], sp[:])
                    recb = smpool.tile([1, w], bf16, tag="recb")
                    nc.vector.tensor_copy(recb[:], rec[:])
                    rbp = pmisc.tile([128, w], f32, tag="misc")
                    nc.tensor.matmul(rbp[:], ones128_bf[:], recb[:])
                    rbs = smpool.tile([128, w], f32, tag="rbs")
                    nc.scalar.copy(rbs[:], rbp[:])
                    return rbs

                def emit_expert(e, acc):
                    # W2 for expert e (its 4 h pairs are ready) + gate-combine.
                    # gs evac on the scalar engine: the DVE FIFO must not gate
                    # the peo/pmisc PSUM rotation (head-of-line blocking).
                    eop = peo.tile([128, w], f32, tag="eo")
                    for dc in range(KC):
                        fc = e * KC + dc
                        nc.tensor.matmul(
                            eop[:],
                            w2_sb[:, e, dc, :],
                            hs[fc // 2][:, fc % 2, :],
                            start=(dc == 0),
                            stop=(dc == KC - 1),
                        )
                    gp = pmisc.tile([128, w], f32, tag="misc")
                    nc.tensor.matmul(gp[:], sel_sb[:, e, :], expt[:])
                    gs = gspool.tile([128, w], f32, tag="gs")
                    nc.scalar.copy(gs[:], gp[:])
                    if e == 0:
                        nc.vector.scalar_tensor_tensor(
                            acc[:], eop[:], b2_sb[:, e : e + 1], gs[:],
                            ALU.add, ALU.mult,
                        )
                    else:
                        tmp = opool.tile([128, w], f32, tag="outt")
                        nc.vector.scalar_tensor_tensor(
                            tmp[:], eop[:], b2_sb[:, e : e + 1], gs[:],
                            ALU.add, ALU.mult,
                        )
                        nc.vector.tensor_add(acc[:], acc[:], tmp[:])

                # --- selection + paired silu (b1 already inside T), with each
                # expert's W2+combine interleaved after its 4th silu pair ---
                next_masks = None
                hs = []
                acc = apool.tile([128, w], f32, tag="acc")
                for pair in range(FC // 2):
                    if pair == 1:
                        rbs = emit_recip_chain()
                    if pair == 9 and i + 1 < nst:
                        next_masks = build_masks(i + 1)
                    hp = pmm.tile([128, 2, ST], f32, tag="mm")
                    for half in range(2):
                        fc = pair * 2 + half
                        for j, (t, vc) in enumerate(chunks):
                            nc.tensor.matmul(
                                hp[:, half, 0:w],
                                t_sb[
                                    :, vc, t, fc // 4,
                                    (fc % 4) * 128 : (fc % 4 + 1) * 128,
                                ],
                                cur_masks[t][vc][:],
                                start=(j == 0),
                                stop=(j == len(chunks) - 1),
                            )
                    h_pair = hpool.tile([128, 2, w], bf16, tag=f"hs{pair}")
                    nc.scalar.activation(h_pair[:], hp[:, :, 0:w], AF.Silu, bias=0.0)
                    hs.append(h_pair)
                    if pair % 4 == 3:
                        emit_expert(pair // 4, acc)
                    if pair == 11 and i + 1 < nst:
                        cur_expt = emit_gating(i + 1, next_masks)

                outt = opool.tile([128, w], f32, tag="outt")
                nc.vector.tensor_tensor(outt[:], acc[:], rbs[:], ALU.mult)
                nc.sync.dma_start(outd[:, _off(i) : _off(i) + w], outt[:])
                if next_masks is not None:
                    cur_masks = next_masks

    if legalize:
        _legalize_waits(nc)
    mybir.codegen_inst_isa_subclasses(nc)
    return nc


def assign_slots(x):
    """Bucket tokens by (i0//128, i1//128) into 16 pure supertiles (512 slots,
    padded) + spill. Returns per-core slot->token maps and n_mixed."""
    x = np.asarray(x)
    slot_maps = []
    spills = []
    for c in range(NCORES):
        xc = x[c * BL : (c + 1) * BL]
        key = (xc[:, 0] // 128) * VC + xc[:, 1] // 128
        order = np.argsort(key, kind="stable")
        ks = key[order]
        slots = np.full(NPURE * ST, -1, dtype=np.int64)
        spill = []
        for b in range(NPURE):
            toks = order[ks == b]
            n = min(len(toks), ST)
            slots[b * ST : b * ST + n] = toks[:n]
            spill.extend(toks[ST:])
        slot_maps.append(slots)
        spills.append(np.array(spill, dtype=np.int64))
    n_mixed = max(
        (len(s) + STM - 1) // STM if len(s) else 0 for s in spills
    )
    full_maps = []
    for c in range(NCORES):
        m = np.full(NPURE * ST + n_mixed * STM, -1, dtype=np.int64)
        m[: NPURE * ST] = slot_maps[c]
        m[NPURE * ST : NPURE * ST + len(spills[c])] = spills[c]
        full_maps.append(m)
    return full_maps, n_mixed


def marshal_inputs(x, emb0, emb1, W1, b1, W2, b2, Wg, bg, slot_maps, n_mixed):
    """Host-side: cast/reshape full inputs into per-core in_maps."""
    x = np.asarray(x)

    xh = {"x0": [], "x1": []}
    for c in range(NCORES):
        m = slot_maps[c]
        xc = x[c * BL : (c + 1) * BL]
        xv = np.zeros((len(m), 2), dtype=np.float16)
        valid = m >= 0
        xv[valid] = xc[m[valid]].astype(np.float16)
        # pad slots: -1 matches no iv entry -> zero one-hot -> output junk
        # that the host discards.
        xv[~valid] = -1.0
        xh["x0"].append(np.ascontiguousarray(xv[:, 0].reshape(1, len(m))))
        xh["x1"].append(np.ascontiguousarray(xv[:, 1].reshape(1, len(m))))

    shared = {}
    # embT[t, kc, p, v] = emb_t[v, kc*128 + p]
    embt = np.stack(
        [np.asarray(e).T.reshape(KC, 128, V) for e in (emb0, emb1)], axis=0
    )
    shared["embt"] = np.ascontiguousarray(embt.astype(BF16))
    # W1flat[k, f] with f = e*1024 + d
    w1flat = np.asarray(W1).transpose(1, 0, 2).reshape(IN, F)
    shared["w1m"] = np.ascontiguousarray(
        w1flat.reshape(2, KC, 128, FB, 1024).transpose(0, 3, 1, 2, 4).astype(BF16)
    )
    shared["b1row"] = np.ascontiguousarray(
        np.asarray(b1).reshape(1, F).astype(BF16)
    )
    shared["bgrow"] = np.ascontiguousarray(
        np.asarray(bg).reshape(1, E).astype(BF16)
    )
    shared["wgm"] = np.ascontiguousarray(
        np.asarray(Wg).reshape(2, KC, 128, E).transpose(2, 0, 1, 3).astype(BF16)
    )
    shared["w2s"] = np.ascontiguousarray(
        np.asarray(W2).reshape(E, KC, 128, OUT).transpose(2, 0, 1, 3).astype(BF16)
    )
    shared["b2s"] = np.ascontiguousarray(np.asarray(b2).T.astype(np.float32))
    shared["sels"] = np.ascontiguousarray(
        np.broadcast_to(np.eye(E, dtype=np.float32)[:, :, None], (E, E, 128)).astype(
            BF16
        )
    )
    shared["ivs"] = np.ascontiguousarray(
        (np.arange(VC)[None, :] * 128 + np.arange(128)[:, None]).astype(np.float32)
    )
    return [
        {**{k: v[c] for k, v in xh.items()}, **shared} for c in range(NCORES)
    ]


def kernel(x, emb0, emb1, W1, b1, W2, b2, Wg, bg):
    global LAST_EXEC_NS
    slot_maps, n_mixed = assign_slots(x)
    nc = build_program(n_mixed)
    in_maps = marshal_inputs(
        x, emb0, emb1, W1, b1, W2, b2, Wg, bg, slot_maps, n_mixed
    )
    trace = os.environ.get("BASSMOE_TRACE", "0") == "1"
    res = run_bass_kernel_spmd(nc, in_maps, list(range(NCORES)), trace=trace)
    LAST_EXEC_NS = res.exec_time_ns
    out = np.empty((B, OUT), dtype=np.float32)
    for c in range(NCORES):
        m = slot_maps[c]
        valid = m >= 0
        r = res.results[c]["out"]  # [128, nst*ST]
        out[c * BL + m[valid], :] = r[:, valid].T
    return out
